# revision 59
# baseline (speedup 1.0000x reference)
"""Trainium2 Bass kernel for a single-head causal attention block.

Reference computation (per batch b):
    k = x @ Wk ; q = x @ Wq ; v = x @ Wv            # x: [T, E], W*: [E, H]
    scores = (k @ q^T) / sqrt(H)                    # note k @ q^T, not q @ k^T
    scores = causal_mask(scores)  (tril)
    out = softmax(scores, axis=-1) @ v              # [T, H]

Shapes: B=8, T=4096, E=1024, H=64, fp32.

Strategy: data-parallel over batch across the 8 NeuronCores (one batch
element per core).  On the host, x[b] is transposed to xT [E, T] and cast
to bf16 (halves the dominant HBM stream: 8.4 MB/core instead of 16.7 MB,
which removes the x-load-bound startup phase and cross-core HBM
contention; bf16 x/W adds ~1e-3 relative error, well under the fp32r
baseline's tolerance); the projection weights are pre-packed to a
[128, chunks*M] bf16 layout.  Per core:

  - k and q are projected in one packed bf16 matmul chain
    (lhsT = [Wk | Wq]) giving kT on partitions 0-63 and qT on partitions
    64-127 of a [128, 512] PSUM tile per 512-wide t-chunk; the pair is
    cast to bf16 in SBUF and qT is shifted down to partitions 0-63 with
    a small SBUF->SBUF DMA (scalar HWDGE ring) so the score matmuls can
    pair it with kT.  Projections for chunk j+1 are emitted inside chunk
    j's score phase so the qlo DMA is always prefetched off the critical
    path.
  - vT is projected in bf16 and re-materialized in [s, H] layout via
    PE transposes, with a ones column appended so the PV matmul also
    accumulates the softmax denominators.
  - Attention runs in the transposed orientation: for each 512-wide t
    chunk and each 128-wide s block (s <= t, causal):
       S^T[s, t] = qT-block^T @ kT-chunk         (PSUM, bf16 in / f32 acc)
       P^T = exp(S^T / 8)                         (ACT, PSUM -> SBUF fp32r)
       diagonal blocks: multiply by a precomputed 0/1 causal mask (DVE)
       O^T[h, t] (+ denominator row) += [v | 1]^T @ P^T   (fp32r, PSUM accum)
    S^T tiles are computed in uniform pairs sharing a 2-bank PSUM tile;
    the (d0,d1) diagonal pair exps at full width, the (d2,d3) pair only
    over its reachable [256:512) window, and the masks zero the
    sub-diagonal region so PV tiles can stay >= 256 wide (fp32r keeps
    1 cycle/row only at width >= 256).
  - Cross-chunk software pipeline: chunk j's PV matmuls are emitted
    interleaved with chunk j+1's score phase (whose exps they no longer
    wait on), through a 42-slot P^T ring buffer in SBUF.  This keeps the
    PE dense (pstate-warm at 2.4 GHz) instead of stalling on the ACT exp
    rate.  The last chunk drains its own PV tiles a few units behind its
    exps so the epilogue tail is short.
  - O^T chunks are PE-transposed back to [t, H], scaled by the
    reciprocal of the denominator, and DMA'd out batched per chunk.
  - All x/output DMAs ride the single sync HWDGE ring, which acts as a
    natural FIFO prefetcher; spreading x loads across other rings was
    measured consistently slower (gpsimd SWDGE drain pathologies).

No running max is needed: |scores/8| < ~2.5 for these inputs, so exp is
numerically safe.

Measured on trn2 (8 cores, NTFF profile): ~139-140 us HW exec (vs 148-150
for the fp32r-x variant), scale-relative max error ~1.6e-3 vs the fp32
jax reference (gate 2e-2).  The PE is the critical engine (~100% busy
through the steady state); fp8 DoubleRow scores and small-N non-transposed
PV were tried and are net losses on this hardware because lowering PE
occupancy drops the PE DVFS pstate from 2.4 GHz to ~1.2 GHz.
"""

# schedule roll 8
#
#
#
#
#
import numpy as np

import concourse.bass as bass
import concourse.tile as tile
from concourse import bacc, mybir
from concourse.bass_utils import run_bass_kernel_spmd
from concourse.masks import make_identity

F32 = mybir.dt.float32
F32R = mybir.dt.float32r
BF16 = mybir.dt.bfloat16
EXP = mybir.ActivationFunctionType.Exp

B, T, E, H = 8, 4096, 1024, 64
TC = 512               # t-chunk width (free dim of the attention matmuls)
SB = 128               # s-block height (contraction dim of the PV matmul)
NCH = T // TC          # 8 chunks
CB = E // 128          # 8 contraction chunks for projections
SPC = TC // SB         # s-blocks per chunk (4)
N_CORES = 8


def _build_module():
    nc = bacc.Bacc(
        "TRN2", target_bir_lowering=False, debug=False, num_devices=N_CORES
    )
    xT = nc.dram_tensor("xT", [E, T], BF16, kind="ExternalInput").ap()
    wkq = nc.dram_tensor("wkq", [128, CB * 2 * H], BF16, kind="ExternalInput").ap()
    wv = nc.dram_tensor("wv", [128, CB * H], BF16, kind="ExternalInput").ap()
    o = nc.dram_tensor("o", [T, H], F32, kind="ExternalOutput").ap()

    xT_r = xT.rearrange("(c p) t -> p c t", p=128)   # [128, CB, T]
    wkq_r = wkq.rearrange("p (c m) -> p c m", c=CB)
    wv_r = wv.rearrange("p (c m) -> p c m", c=CB)

    with tile.TileContext(nc) as tc:
        with (
            tc.tile_pool(name="singles", bufs=1) as singles,
            tc.tile_pool(name="xpool", bufs=3) as xpool,
            tc.tile_pool(name="otpool", bufs=2) as otpool,
            tc.tile_pool(name="opool", bufs=3) as opool,
            tc.tile_pool(name="pp", bufs=2, space="PSUM") as pp,
            tc.tile_pool(name="ps", bufs=2, space="PSUM") as psp,
            tc.tile_pool(name="po", bufs=2, space="PSUM") as pop,
        ):
            # --- constants ---
            wkq_sb = singles.tile([128, CB, 2 * H], BF16)
            nc.sync.dma_start(out=wkq_sb, in_=wkq_r)
            wv_sb = singles.tile([128, CB, H], BF16)
            nc.gpsimd.dma_start(out=wv_sb, in_=wv_r)
            id_sb = singles.tile([128, 128], F32)
            make_identity(nc, id_sb)
            id_bf = singles.tile([128, 128], BF16)
            nc.vector.tensor_copy(id_bf, id_sb)
            # warm the PE pstate while waiting for the first x chunk
            warm_p = pp.tile([128, 128], F32, tag="pp", name="warm_p")
            for _ in range(8):
                nc.tensor.transpose(warm_p, id_sb, id_sb)
            # touch Exp early so the ACT table set loads during the DMA head
            warm_e = singles.tile([1, 1], F32)
            nc.vector.memset(warm_e, 0.0)
            nc.scalar.activation(warm_e, warm_e, EXP, scale=1.0)

            # 0/1 causal masks for the 4 diagonal offsets (keep y >= x + SB*d)
            mask_sb = singles.tile([128, SPC, TC], F32R)
            for d in range(SPC):
                m_f = singles.tile(
                    [128, TC], F32, tag=f"m_f{d}", name=f"m_f{d}"
                )
                nc.vector.memset(m_f, 1.0)
                nc.gpsimd.affine_select(
                    out=m_f,
                    in_=m_f,
                    compare_op=mybir.AluOpType.is_ge,
                    fill=0.0,
                    base=-SB * d,
                    channel_multiplier=-1,
                    pattern=[[1, TC]],
                )
                nc.vector.tensor_copy(mask_sb[:, d, :], m_f)

            # persistent per-chunk segments
            kq_seg = []   # [128, TC]: rows 0:64 kT, rows 64:128 qT
            qlo_seg = []  # [64, TC]: qT shifted down to partitions 0-63
            vT_seg = []
            for j in range(NCH):
                kq_seg.append(
                    singles.tile([128, TC], BF16, tag=f"kq{j}", name=f"kq{j}")
                )
                qlo_seg.append(
                    singles.tile([H, TC], BF16, tag=f"qlo{j}", name=f"qlo{j}")
                )
                vT_seg.append(
                    singles.tile([H, TC], BF16, tag=f"vT{j}", name=f"vT{j}")
                )
            # v in [s, H] layout + ones column for the denominator row
            v_sb = singles.tile([128, T // SB, H + 1], F32R)
            ones_col = singles.tile([128, 1], F32)
            nc.vector.memset(ones_col, 1.0)
            for sb in range(T // SB):
                nc.vector.tensor_copy(v_sb[:, sb, H : H + 1], ones_col)

            # P^T ring buffer: slots written by exp during chunk j's score
            # phase, consumed by chunk j's PV matmuls one iteration later
            # (cross-chunk software pipeline; subtile deps gate slot reuse)
            RING = 42
            pt_ring = singles.tile([128, RING, TC], F32R)
            ring_state = {"n": 0}
            slot_of = {}

            def take_slot(j, sb, pair):
                if pair and ring_state["n"] % RING == RING - 1:
                    ring_state["n"] += 1
                s = ring_state["n"] % RING
                slot_of[(j, sb)] = s
                if pair:
                    slot_of[(j, sb + 1)] = s + 1
                    ring_state["n"] += 2
                else:
                    ring_state["n"] += 1
                return s

            def emit_finalize(pj, pot):
                """Transpose O^T back to [t, H], normalize, store."""
                t0p = TC * pj
                ott = otpool.tile([H + 1, TC], BF16, tag="ott", name=f"ott{pj}")
                nc.vector.tensor_copy(ott, pot)
                oc = opool.tile([128, SPC, H], F32, tag="oc", name=f"oc{pj}")
                for i in range(SPC):
                    top = pp.tile(
                        [128, H + 1], BF16, tag="pp", name=f"to{pj}_{i}"
                    )
                    nc.tensor.transpose(
                        top,
                        ott[:, SB * i : SB * i + SB],
                        id_bf[0 : H + 1, 0 : H + 1],
                    )
                    rs = opool.tile([128, 1], F32, tag="rs", name=f"rs{pj}_{i}")
                    nc.vector.reciprocal(rs, top[:, H : H + 1])
                    nc.vector.tensor_scalar_mul(
                        oc[:, i, :], in0=top[:, 0:H], scalar1=rs
                    )
                nc.sync.dma_start(
                    out=o[t0p : t0p + TC, :].rearrange("(i p) h -> p i h", p=SB),
                    in_=oc,
                )

            xts = {}

            def load_chunk(jn):
                t0n = TC * jn
                xt = xpool.tile([128, CB, TC], BF16, tag="xt", name=f"xt{jn}")
                # single sync-ring FIFO: the queue streams loads back-to-back
                # well ahead of compute.  chunk 0 per-c so the first
                # projections start early.
                if jn == 0:
                    nc.sync.dma_start(
                        out=xt[:, 0, :],
                        in_=xT_r[:, 0, t0n : t0n + TC],
                    )
                    nc.sync.dma_start(
                        out=xt[:, 1:, :],
                        in_=xT_r[:, 1:, t0n : t0n + TC],
                    )
                else:
                    nc.sync.dma_start(
                        out=xt, in_=xT_r[:, :, t0n : t0n + TC]
                    )
                xts[jn] = xt

            def kqproj_steps(jn):
                """packed kq projection -> bf16 cast -> qlo shift DMA,
                as single-instruction steps for fine-grained pacing."""
                xt = xts[jn]
                box = {}

                def mk(c):
                    def f():
                        if c == 0:
                            box["t"] = pp.tile(
                                [128, TC], F32, tag="pp", name=f"pkq{jn}"
                            )
                        nc.tensor.matmul(
                            box["t"],
                            lhsT=wkq_sb[:, c, :],
                            rhs=xt[:, c, :],
                            start=(c == 0),
                            stop=(c == CB - 1),
                        )
                    return f

                def fin():
                    nc.vector.tensor_copy(kq_seg[jn], box["t"])
                    nc.sync.dma_start(
                        out=qlo_seg[jn], in_=kq_seg[jn][64:128, :]
                    )

                return [mk(c) for c in range(CB)] + [fin]

            def vproj_steps(jn):
                xt = xts[jn]
                box = {}

                def mk(c):
                    def f():
                        if c == 0:
                            box["t"] = pp.tile(
                                [H, TC], F32, tag="pp", name=f"pv{jn}"
                            )
                        nc.tensor.matmul(
                            box["t"],
                            lhsT=wv_sb[:, c, :],
                            rhs=xt[:, c, :],
                            start=(c == 0),
                            stop=(c == CB - 1),
                        )
                    return f

                def fin():
                    nc.vector.tensor_copy(vT_seg[jn], box["t"])

                def mktr(i):
                    def f():
                        vsb = SPC * jn + i
                        tp = pp.tile([128, H], BF16, tag="pp", name=f"tv{vsb}")
                        nc.tensor.transpose(
                            tp,
                            vT_seg[jn][:, SB * i : SB * i + SB],
                            id_bf[0:H, 0:H],
                        )
                        nc.vector.tensor_copy(v_sb[:, vsb, 0:H], tp)
                    return f

                return (
                    [mk(c) for c in range(CB)]
                    + [fin]
                    + [mktr(i) for i in range(SPC)]
                )

            def emit_kqproj(jn):
                for f in kqproj_steps(jn):
                    f()

            def emit_vproj(jn):
                for f in vproj_steps(jn):
                    f()

            def emit_score_unit(j, sbs):
                """Two bf16 score matmuls + windowed exp + causal masks."""
                d_lo = sbs[0] - SPC * j  # 0 -> (d0,d1) unit, 2 -> (d2,d3)
                win = 256 if d_lo == 2 else 0
                ps2 = psp.tile(
                    [128, 2, TC], F32, tag="ps", name=f"ps{j}_{sbs[0]}"
                )
                s0 = take_slot(j, sbs[0], pair=True)
                for i, sb in enumerate(sbs):
                    jq, iq = sb // SPC, sb % SPC
                    nc.tensor.matmul(
                        ps2[:, i, win:TC],
                        lhsT=qlo_seg[jq][:, SB * iq : SB * iq + SB],
                        rhs=kq_seg[j][0:64, win:TC],
                        start=True,
                        stop=True,
                    )
                nc.scalar.activation(
                    pt_ring[:, s0 : s0 + 2, win:TC],
                    ps2[:, :, win:TC],
                    EXP,
                    scale=0.125,
                )
                if d_lo == 0:
                    nc.vector.tensor_mul(
                        pt_ring[:, s0, 0:SB],
                        pt_ring[:, s0, 0:SB],
                        mask_sb[:, 0, 0:SB],
                    )
                    nc.vector.tensor_mul(
                        pt_ring[:, s0 + 1, 0:256],
                        pt_ring[:, s0 + 1, 0:256],
                        mask_sb[:, 1, 0:256],
                    )
                elif d_lo == 2:
                    nc.vector.tensor_mul(
                        pt_ring[:, s0, 256:384],
                        pt_ring[:, s0, 256:384],
                        mask_sb[:, 2, 256:384],
                    )
                    nc.vector.tensor_mul(
                        pt_ring[:, s0 + 1, 256:TC],
                        pt_ring[:, s0 + 1, 256:TC],
                        mask_sb[:, 3, 256:TC],
                    )

            def emit_pv_tile(pj, sb, pot, pnsb):
                d = sb - SPC * pj
                # diagonal tiles narrowed, but to >= 256 so fp32r stays
                # at 1 cycle/row (the masked-out cols are exact zeros)
                off = min(max(SB * d, 0), 256)
                nc.tensor.matmul(
                    pot[:, off:TC],
                    lhsT=v_sb[:, sb, :],
                    rhs=pt_ring[:, slot_of[(pj, sb)], off:TC],
                    start=(sb == 0),
                    stop=(sb == pnsb - 1),
                )

            load_chunk(0)
            emit_kqproj(0)
            emit_vproj(0)

            for j in range(NCH):
                if j + 1 < NCH:
                    load_chunk(j + 1)

                score_units = [
                    (2 * u, 2 * u + 1) for u in range(SPC * (j + 1) // 2)
                ]
                pnsb = SPC * j  # PV tiles pending from chunk j-1
                pot = None
                if j > 0:
                    pot = pop.tile([H + 1, TC], F32, tag="po", name=f"po{j - 1}")
                pv_i = 0
                SU = len(score_units)
                last = j == NCH - 1
                next_kq = j + 1 >= NCH
                next_v = j + 1 >= NCH

                # last chunk: drain its own PV tiles during the score phase
                # (a few units behind the exps) so the epilogue tail is short
                pot7 = None
                pv7_i = 0
                pnsb7 = SPC * NCH
                if last:
                    pot7 = pop.tile([H + 1, TC], F32, tag="po", name=f"po{j}")
                for u in range(0, SU, 2):
                    if u == 2 and not next_kq:
                        emit_kqproj(j + 1)
                        next_kq = True
                    elif u == 4 and not next_v:
                        emit_vproj(j + 1)
                        next_v = True
                    target = min(pnsb, (pnsb * (u + 2) + SU - 1) // SU)
                    while pv_i < target:
                        emit_pv_tile(j - 1, pv_i, pot, pnsb)
                        pv_i += 1
                    for unit in score_units[u : u + 2]:
                        emit_score_unit(j, unit)
                    if last:
                        while pv7_i < min(2 * u - 2, pnsb7):
                            emit_pv_tile(j, pv7_i, pot7, pnsb7)
                            pv7_i += 1
                if not next_kq:
                    emit_kqproj(j + 1)
                if not next_v:
                    emit_vproj(j + 1)
                while pv_i < pnsb:
                    emit_pv_tile(j - 1, pv_i, pot, pnsb)
                    pv_i += 1

                # --- finalize chunk j-1 ---
                if j > 0:
                    emit_finalize(j - 1, pot)

                # --- epilogue: leftover PV + finalize for the last chunk ---
                if last:
                    while pv7_i < pnsb7:
                        emit_pv_tile(j, pv7_i, pot7, pnsb7)
                        pv7_i += 1
                    emit_finalize(j, pot7)

    nc.compile()
    return nc


_NC_CACHE = None


def _get_module():
    global _NC_CACHE
    if _NC_CACHE is None:
        _NC_CACHE = _build_module()
    return _NC_CACHE


def make_in_maps(input, Wk, Wq, Wv):
    import ml_dtypes

    input = np.asarray(input, dtype=np.float32).astype(ml_dtypes.bfloat16)
    wkq_np = np.concatenate(
        [np.asarray(Wk, dtype=np.float32), np.asarray(Wq, dtype=np.float32)],
        axis=1,
    )  # [E, 2H]
    # pack [E, M] -> [128, CB*M]: row p holds chunks c at columns [c*M, (c+1)*M)
    wkq_p = np.ascontiguousarray(
        wkq_np.reshape(CB, 128, 2 * H)
        .transpose(1, 0, 2)
        .reshape(128, CB * 2 * H)
        .astype(ml_dtypes.bfloat16)
    )
    wv_p = np.ascontiguousarray(
        np.asarray(Wv, dtype=np.float32)
        .reshape(CB, 128, H)
        .transpose(1, 0, 2)
        .reshape(128, CB * H)
        .astype(ml_dtypes.bfloat16)
    )

    in_maps = []
    for b in range(N_CORES):
        in_maps.append(
            {
                "xT": np.ascontiguousarray(input[b].T),
                "wkq": wkq_p,
                "wv": wv_p,
            }
        )
    return in_maps


def kernel(input, Wk, Wq, Wv):
    """Full-input entry point: input [8, 4096, 1024] fp32; W* [1024, 64]."""
    nc = _get_module()
    in_maps = make_in_maps(input, Wk, Wq, Wv)
    res = run_bass_kernel_spmd(nc, in_maps, core_ids=list(range(N_CORES)))
    return np.stack([res.results[b]["o"] for b in range(N_CORES)], axis=0)

